# revision 41
# baseline (speedup 1.0000x reference)
"""CSPNGenerate Trainium2 kernel (v2: f16 row-duplicated slab).

Per core (8 cores = batch b in 0..3  x  half in 0..1):
  input slab  [128, 11, 18*1218] f16: per 16-row block, 18 row-slots;
              partitions 0:64 = channels of input row (base+j-1)  ("h0"),
              partitions 64:128 = channels of input row (base+j)  ("h1"),
              so a 128-partition column at slot j carries TWO adjacent rows.
  output      [9, 176, 1216] f16  (9 planes; plane 4 holds -R*T, host adds 1)

Conv restructure: for output row r, taps di=0,1 contract in ONE 128-partition
matmul (slots j=r..r+1 read rows r-1,r in h0/h1), tap di=2 in a 64-partition
matmul (h0 slot r+2 for bank-A groups, h1 slot r+1 for bank-B groups; the two
64-part tiles co-issue on opposite PE row halves). 6 matmul rows/pixel vs 9.

PSUM packing: 4 row-pair groups per bank at column tile positions 32g, so one
ACT instruction evacuates a whole bank (bias from b9x; garbage partitions get
bias 0 on top of once-zeroed PSUM => exact zeros, no per-chunk memsets).
abs on Pool (tensor_tensor abs_max), reciprocal on ACT (table), selector
matmuls (S-reduce + R-broadcast) as in v1. Output staged/DMA'd in f16.
"""

import sys

if "/opt/trn_rl_repo" not in sys.path:
    sys.path.insert(0, "/opt/trn_rl_repo")

import numpy as np
import concourse.bass as bass
import concourse.mybir as mybir
from concourse.tile import TileContext
from concourse.vector_clock import ScopedClock, VectorClock


# ---- toolchain workarounds (drain-wait split, per-instruction sync-wait
# limit, optional NTFF profiling shim) ----
def _drain_and_barrier_split(self, tick_clock, wait_clock):
    gclock = tick_clock.global_clock
    nprocs = len(gclock)
    # One NOP per nonzero proc tick; add_sem_waits elides already-observed
    # ticks, so each NOP carries at most one wait.
    for proc in range(nprocs):
        tick = gclock[proc]
        if tick <= 0:
            continue
        vc = VectorClock([0] * nprocs)
        vc.require_at_least(proc, tick)
        nop = self.nc.sync.nop(nofuse=True, hint="drain_split_wait")
        wait_clock.add_sem_waits(nop.ins, ScopedClock({None: vc}))

    # All waits were attached to the NOPs above (same engine, program order),
    # so the drain itself needs none — keeping it under the CoreV3 codegen
    # limit on sync-wait commands for Drain.
    self.nc.sync.drain()

    self.nc.all_engine_barrier()
    assert self.sems is not None
    popped = self.nc._tile_sem_poison_stack.pop()
    assert popped is self._sem_poison
    self.nc.clear_and_free_semaphores(list(self.sems.allocated().values()))
    self.nc.all_engine_barrier()


def install():
    TileContext._drain_and_barrier = _drain_and_barrier_split
    install_wait_split()


_MAX_WAITS = 1


def _split_waits_json(bir: bytes) -> bytes:
    """Walrus in this toolchain rejects instructions carrying more than one
    sync-wait command ("Too many sync wait commands"). Move excess waits onto
    same-engine NoOps inserted immediately before the instruction."""
    import orjson

    m = orjson.loads(bir)
    for func in m.get("functions", []):
        for block in func.get("blocks", []):
            out = []
            changed = False
            for inst in block["instructions"]:
                si = inst.get("sync_info") or {}
                waits = si.get("on_wait") or []
                if len(waits) > _MAX_WAITS:
                    keep = waits[-_MAX_WAITS:]
                    extra = waits[:-_MAX_WAITS]
                    for k, w in enumerate(extra):
                        out.append(
                            {
                                "debug": inst.get("debug", 0),
                                "engine": inst["engine"],
                                "ins": [],
                                "name": f"{inst['name']}-wsplit{k}",
                                "opcode": "NoOp",
                                "outs": [],
                                "sync_info": {"on_update": [], "on_wait": [w]},
                                "text_hint": "wait_split",
                            }
                        )
                    si["on_wait"] = keep
                    inst["sync_info"] = si
                    changed = True
                out.append(inst)
            if changed:
                block["instructions"] = out
    return orjson.dumps(m)


def install_wait_split():
    import concourse.bass as _bass

    if getattr(_bass.Bass, "_wait_split_installed", False):
        return
    orig = _bass.Bass.to_json_bytes

    def to_json_bytes(self):
        return _split_waits_json(orig(self))

    _bass.Bass.to_json_bytes = to_json_bytes
    _bass.Bass._wait_split_installed = True


def install_ntff_shim():
    """Provide the missing ``antenv.axon_hooks`` module so trace=True works
    under axon, wiring it to trn_boot's ctypes NTFF hook factory."""
    import sys
    import types

    if "antenv.axon_hooks" in sys.modules:
        return
    mod = types.ModuleType("antenv.axon_hooks")
    state = {"hook": None}

    def set_axon_ntff_profile_hook(h):
        state["hook"] = h

    def get_axon_ntff_profile_hook():
        return state["hook"]

    mod.set_axon_ntff_profile_hook = set_axon_ntff_profile_hook
    mod.get_axon_ntff_profile_hook = get_axon_ntff_profile_hook
    sys.modules["antenv.axon_hooks"] = mod

    try:
        from trn_agent_boot.trn_boot import _ntff_profile_via_ctypes

        hook = _ntff_profile_via_ctypes("/opt/axon/libaxon_pjrt.so")
        if hook is not None:
            set_axon_ntff_profile_hook(hook)
    except Exception as e:  # profiling optional — degrade to no trace
        print(f"ntff shim: hook install failed: {e}")

    # upload_artifacts pushes to a remote bucket that doesn't exist in this
    # container; stub it so trace post-processing stays local.
    from concourse import bass_utils

    bass_utils.upload_artifacts = lambda tmpdir: tmpdir


# geometry
B, C, H, W, K = 4, 64, 352, 1216, 3
OC = 8
HALF = 176  # rows per core
WP = W + 2  # padded width
TR = 16  # output rows per block
SLOT = TR + 2  # row-slots per block slab
NBLK = HALF // TR  # 11
XCH = [(0, 256), (256, 256), (512, 256), (768, 256), (1024, 192)]
MROW = [0, 1, 2, 3, 8, 4, 5, 6, 7]  # kernel row m for output plane p

F32 = mybir.dt.float32
F32R = mybir.dt.float32r
F16 = mybir.dt.float16
AF = mybir.ActivationFunctionType


def build_nc():
    nc = bass.Bass()
    slab = nc.dram_tensor("slab", [64, NBLK, SLOT * WP], F16, kind="ExternalInput")
    w6 = nc.dram_tensor("w6", [128, 54], F16, kind="ExternalInput")
    b9x = nc.dram_tensor("b9x", [128, 1], F32, kind="ExternalInput")
    ssel = nc.dram_tensor("ssel", [128, 18], F16, kind="ExternalInput")
    bsel = nc.dram_tensor("bsel", [128, 1024], F32, kind="ExternalInput")
    out = nc.dram_tensor("out", [9, HALF, W], F16, kind="ExternalOutput")

    with TileContext(nc) as tc:
        with (
            tc.tile_pool(name="consts", bufs=1) as cpool,
            tc.tile_pool(name="slabp", bufs=2) as slabp,
            tc.tile_pool(name="work", bufs=6) as work,
            tc.tile_pool(name="stagep", bufs=2) as stagep,
            tc.tile_pool(name="pcA", bufs=2, space="PSUM") as pcA,
            tc.tile_pool(name="pcB", bufs=2, space="PSUM") as pcB,
            tc.tile_pool(name="ps_s8", bufs=2, space="PSUM") as ps_s8,
            tc.tile_pool(name="ps_rbc", bufs=1, space="PSUM") as ps_rbc,
        ):
            w6t = cpool.tile([128, 54], F16, name="w6t")
            b9xt = cpool.tile([128, 1], F32, name="b9xt")
            sselt = cpool.tile([128, 18], F16, name="sselt")
            bselt = cpool.tile([128, 1024], F32R, name="bselt")
            nc.gpsimd.dma_start(w6t[:], w6[:])
            nc.gpsimd.dma_start(b9xt[:], b9x[:])
            nc.gpsimd.dma_start(sselt[:], ssel[:])
            nc.gpsimd.dma_start(bselt[:], bsel[:])

            # One-time zero of the conv PSUM banks: matmuls only ever write
            # partitions 32g..32g+9, the evacuation reads all 128, and the
            # garbage partitions must be exactly 0.0 (bias b9x is 0 there).
            for pool, nm in ((pcA, "cA"), (pcB, "cB")):
                for _ in range(2):
                    zt = pool.tile([128, 512], F32, name=nm)
                    nc.vector.memset(zt[:], 0.0)
            # S banks start at 1.0 so unused partitions reciprocate to a
            # finite 1.0 (they hit zero bsel weights; 0*inf would be NaN)
            for _ in range(2):
                zs = ps_s8.tile([128, 512], F32, name="s8")
                nc.vector.memset(zs[:], 1.0)

            # Software pipeline (as v1): each chunk's selector matmuls are
            # deferred behind the NEXT chunk's conv (stage 1: S-matmul +
            # reciprocal) and 4 chunks later (stage 2: broadcast matmul +
            # multiplies + output DMAs) to keep the in-order PE queue dense.
            from collections import deque

            pending = deque()

            # S-sums of 4 consecutive chunks (phases) share one PSUM bank at
            # partition offsets 32*ph, so ONE [128, n] DVE reciprocal serves
            # 4 chunks (the reciprocal's cost depends on columns, not rows).
            group_state = {}

            def emit_recip():
                s8x = group_state.pop("s8x", None)
                if s8x is None:
                    return
                r8x = work.tile([128, 512], F32R, name="r8")
                with nc.allow_low_precision(reason="f32r normalize"):
                    nc.vector.reciprocal(r8x[:, 0:256], s8x[:, 0:256])
                    nc.vector.reciprocal(r8x[:, 256:512], s8x[:, 256:512])
                for stt_i in pending:
                    if "r8x" not in stt_i:
                        stt_i["r8x"] = r8x

            def stage1(stt):
                n = stt["n"]
                ph = stt["cno"] % 4
                stt["ph"] = ph
                if ph == 0:
                    group_state["s8x"] = ps_s8.tile([128, 512], F32, name="s8")
                s8x = group_state["s8x"]
                for h in range(2):
                    nc.tensor.matmul(
                        out=s8x[32 * ph : 32 * ph + 9, 0:n],
                        lhsT=sselt[:, 9 * h : 9 * h + 9],
                        rhs=stt["aas"][h][:, 0:n],
                        start=(h == 0),
                        stop=(h == 1),
                        tile_position=(0, 32 * ph),
                    )
                if ph == 3:
                    emit_recip()

            def stage2(stt):
                n, w, x0, ph = stt["n"], stt["w"], stt["x0"], stt["ph"]
                rbcs = [
                    ps_rbc.tile([128, 512], F32, name="rbcA"),
                    ps_rbc.tile([128, 512], F32, name="rbcB"),
                ]
                for h in range(2):
                    nc.tensor.matmul(
                        out=rbcs[h][:, 0:n],
                        lhsT=bselt[:, 256 * ph + 128 * h : 256 * ph + 128 * h + 128],
                        rhs=stt["r8x"][:, 0:n],
                        start=True,
                        stop=True,
                        tile_position=(0, 0),
                    )
                # stage per-partition layout (h, r, x): 2 x 2 x W
                sv = stt["stage"][:].rearrange("p (h r x) -> p h r x", h=2, r=2, x=W)
                for h in range(2):
                    nc.vector.tensor_mul(
                        sv[:, h, :, x0 : x0 + w],
                        stt["ys"][h][:, 0:n].rearrange("p (r x) -> p r x", r=2, x=w),
                        rbcs[h][:, 0:n].rearrange("p (r x) -> p r x", r=2, x=w),
                    )
                if stt["last_chunk"]:
                    blk = stt["blk"]
                    for p in range(9):
                        m = MROW[p]
                        sb = stt["stage"][:].rearrange(
                            "(g m) (h r x) -> g m h r x", g=4, m=32, h=2, r=2, x=W
                        )[:, m, :, :, :]
                        dr = out[p].rearrange(
                            "(a h g r) w -> a g h r w", a=NBLK, h=2, g=4, r=2
                        )[blk]
                        nc.sync.dma_start(dr, sb)

            def advance(newstate):
                if len(pending) >= 1:
                    stage1(pending[-1])
                if len(pending) >= 4:
                    stage2(pending.popleft())
                if newstate is not None:
                    pending.append(newstate)

            def load_block(st_t, blk):
                """HBM load of the undup 64-channel half (split in two so the
                on-chip dedup copies overlap the second load), then SB->SB
                copies building partitions 64:128 = same rows shifted by one
                slot (h1 slot j = h0 slot j+1)."""
                h1 = 9 * WP
                nc.gpsimd.dma_start(st_t[0:64, 0:h1], slab[:, blk, 0:h1])
                nc.gpsimd.dma_start(st_t[0:64, h1 : SLOT * WP], slab[:, blk, h1:])
                nc.gpsimd.dma_start(st_t[64:128, 0 : 8 * WP], st_t[0:64, WP : 9 * WP])
                nc.gpsimd.dma_start(
                    st_t[64:128, 8 * WP : 17 * WP], st_t[0:64, 9 * WP : 18 * WP]
                )

            st = slabp.tile([128, SLOT * WP], F16, name="st")
            load_block(st, 0)
            for blk in range(NBLK):
                # 2D view: [128, SLOT, WP]
                stv = st[:].rearrange("p (r w) -> p r w", r=SLOT, w=WP)
                stage = stagep.tile([128, 4 * W], F16, name="stageAB")
                for ci, (x0, w) in enumerate(XCH):
                    n = 2 * w  # elems per row-pair (2 rows of w)
                    cA = pcA.tile([128, 512], F32, name="cA")
                    cB = pcB.tile([128, 512], F32, name="cB")
                    banks = (cA, cB)
                    # pass 1: taps di=0,1 via one 128-part matmul per pair
                    for dj in range(3):
                        for g8 in range(8):
                            h, g = divmod(g8, 4)
                            j = 8 * h + 2 * g
                            cv = banks[h][32 * g : 32 * g + 9, 0:n].rearrange(
                                "p (r x) -> p r x", r=2, x=w
                            )
                            nc.tensor.matmul(
                                out=cv,
                                lhsT=w6t[:, 9 * dj : 9 * dj + 9],
                                rhs=stv[:, j : j + 2, x0 + dj : x0 + dj + w],
                                start=(dj == 0),
                                stop=False,
                                tile_position=(0, 32 * g),
                            )
                    # pass 2: tap di=2 via 64-part matmuls; bank A reads h0
                    # (slots j+2..j+3), bank B reads h1 (slots j+1..j+2) so
                    # adjacent instructions co-issue on opposite PE halves
                    # and write different PSUM banks.
                    for dj in range(3):
                        for g in range(4):
                            for h in range(2):
                                j = 8 * h + 2 * g
                                cv = banks[h][32 * g : 32 * g + 9, 0:n].rearrange(
                                    "p (r x) -> p r x", r=2, x=w
                                )
                                if h == 0:
                                    rhs = stv[0:64, j + 2 : j + 4, x0 + dj : x0 + dj + w]
                                    lhsT = w6t[0:64, 27 + 9 * dj : 36 + 9 * dj]
                                    tp = (0, 32 * g)
                                else:
                                    rhs = stv[
                                        64:128, j + 1 : j + 3, x0 + dj : x0 + dj + w
                                    ]
                                    lhsT = w6t[64:128, 27 + 9 * dj : 36 + 9 * dj]
                                    tp = (64, 32 * g)
                                nc.tensor.matmul(
                                    out=cv,
                                    lhsT=lhsT,
                                    rhs=rhs,
                                    start=False,
                                    stop=(dj == 2),
                                    tile_position=tp,
                                )

                    newstate = {
                        "n": n,
                        "w": w,
                        "x0": x0,
                        "stage": stage,
                        "last_chunk": ci == len(XCH) - 1,
                        "blk": blk,
                        "cno": blk * len(XCH) + ci,
                    }
                    ys = [
                        work.tile([128, 512], F32, name="yA"),
                        work.tile([128, 512], F32, name="yB"),
                    ]
                    aas = [
                        work.tile([128, 512], F16, name="aA"),
                        work.tile([128, 512], F16, name="aB"),
                    ]
                    newstate["ys"] = ys
                    newstate["aas"] = aas
                    advance(newstate)

                    if ci == 0 and blk + 1 < NBLK:
                        st_next = slabp.tile([128, SLOT * WP], F16, name="st")
                        load_block(st_next, blk + 1)

                    # evacuate whole banks: y = psum + bias (garbage rows 0),
                    # then |y| on Pool for the S-selector contraction
                    for h in range(2):
                        nc.scalar.activation(
                            ys[h][:, 0:n],
                            banks[h][:, 0:n],
                            AF.Identity,
                            bias=b9xt[:, 0:1],
                            scale=1.0,
                        )
                    for h in range(2):
                        nc.scalar.activation(
                            aas[h][:, 0:n],
                            banks[h][:, 0:n],
                            AF.Abs,
                            bias=b9xt[:, 0:1],
                            scale=1.0,
                        )
                st = st_next
            # drain the pipeline
            if pending:
                stage1(pending[-1])
            emit_recip()  # reciprocal for a trailing partial phase-group
            while pending:
                stage2(pending.popleft())
    return nc


def make_consts(conv_w, gamma, beta, mean, var):
    eps = 1e-5
    s = gamma.astype(np.float64) / np.sqrt(var.astype(np.float64) + eps)
    bt = beta.astype(np.float64) - mean.astype(np.float64) * s
    wp = conv_w.astype(np.float64) * s[:, None, None, None]  # [8, 64, 3, 3]
    w9 = np.concatenate([wp, wp.sum(axis=0, keepdims=True)])  # [9, 64, 3, 3]

    w6 = np.zeros((128, 54), np.float16)
    for dj in range(3):
        # pass1: partitions 0:64 multiply tap di=0, 64:128 tap di=1
        w6[0:64, 9 * dj : 9 * dj + 9] = w9[:, :, 0, dj].T.astype(np.float16)
        w6[64:128, 9 * dj : 9 * dj + 9] = w9[:, :, 1, dj].T.astype(np.float16)
        # pass2: tap di=2 weights replicated in both halves
        w6[0:64, 27 + 9 * dj : 36 + 9 * dj] = w9[:, :, 2, dj].T.astype(np.float16)
        w6[64:128, 27 + 9 * dj : 36 + 9 * dj] = w9[:, :, 2, dj].T.astype(np.float16)

    bt9 = np.concatenate([bt, [bt.sum()]]).astype(np.float32)  # [9]
    b9x = np.zeros((128, 1), np.float32)
    for g in range(4):
        b9x[32 * g : 32 * g + 9, 0] = bt9

    # ssel cols 0:9 (bank A): group-g channel rows -> S row g; cols 9:18
    # (bank B): -> S row 4+g. Col 8 is a dummy 9th output (the dst-partition
    # ISA check wants the same 9-wide shape as the conv matmuls); it gets a
    # copy of S row 0 so its reciprocal stays finite.
    ssel = np.zeros((128, 18), np.float16)
    for g in range(4):
        ssel[32 * g : 32 * g + 8, g] = 1.0
        ssel[32 * g : 32 * g + 8, 9 + 4 + g] = 1.0
    ssel[0:8, 8] = 1.0
    # bsel, per phase ph (column block 256*ph), contracting all 128 R rows:
    # R row 32ph+g -> rows 32g+c (+1 for c<8, -1 for c=8) for bank A (h=0,
    # cols +0:128) and R row 32ph+4+g likewise for bank B (cols +128:256).
    # All other rows have zero weight.
    bsel = np.zeros((128, 1024), np.float32)
    for ph in range(4):
        for g in range(4):
            bsel[32 * ph + g, 256 * ph + 32 * g : 256 * ph + 32 * g + 8] = 1.0
            bsel[32 * ph + g, 256 * ph + 32 * g + 8] = -1.0
            bsel[32 * ph + 4 + g, 256 * ph + 128 + 32 * g : 256 * ph + 128 + 32 * g + 8] = 1.0
            bsel[32 * ph + 4 + g, 256 * ph + 128 + 32 * g + 8] = -1.0
    return w6, b9x, ssel, bsel


TRACE = False
LAST_EXEC_NS = None


def kernel(feature, conv_w, gamma, beta, mean, var, kernel_size):
    global LAST_EXEC_NS
    install()
    if TRACE:
        install_ntff_shim()

    from concourse.bass_utils import run_bass_kernel_spmd

    feature = np.asarray(feature, np.float32)
    conv_w = np.asarray(conv_w, np.float32)
    gamma = np.asarray(gamma, np.float32)
    beta = np.asarray(beta, np.float32)
    mean = np.asarray(mean, np.float32)
    var = np.asarray(var, np.float32)

    w6, b9x, ssel, bsel = make_consts(conv_w, gamma, beta, mean, var)

    # padded feature [B, C, H+3, W+2] in f16 (one extra zero row at the
    # bottom so the unused h1 slot 17 of the last block stays in range)
    fpad = np.zeros((B, C, H + 3, WP), np.float16)
    fpad[:, :, 1 : H + 1, 1 : W + 1] = feature

    in_maps = []
    for core in range(8):
        b, half = core // 2, core % 2
        h0 = half * HALF
        slab2 = np.empty((64, NBLK, SLOT * WP), np.float16)
        for k in range(NBLK):
            base = h0 + TR * k  # fpad row of slot 0 for the h0 half
            slab2[:, k, :] = fpad[b, :, base : base + SLOT, :].reshape(C, -1)
        in_maps.append(
            {
                "slab": slab2,
                "w6": w6,
                "b9x": b9x,
                "ssel": ssel,
                "bsel": bsel,
            }
        )

    nc = build_nc()
    res = run_bass_kernel_spmd(nc, in_maps, core_ids=list(range(8)), trace=TRACE)
    LAST_EXEC_NS = res.exec_time_ns

    out_full = np.zeros((B, 9, H + 2, WP), np.float32)
    for core in range(8):
        b, half = core // 2, core % 2
        h0 = half * HALF
        r = np.asarray(res.results[core]["out"], dtype=np.float32)  # [9, 176, 1216]
        for p in range(9):
            i, j = p // 3, p % 3
            plane = r[p]
            if p == 4:
                plane = 1.0 + plane
            out_full[b, p, h0 + i : h0 + HALF + i, j : j + W] = plane
    return out_full


# revision 48
# speedup vs baseline: 1.2127x; 1.2127x over previous
"""CSPNGenerate Trainium2 kernel (v2: f16 row-duplicated slab).

Per core (8 cores = batch b in 0..3  x  half in 0..1):
  input slab  [128, 11, 18*1218] f16: per 16-row block, 18 row-slots;
              partitions 0:64 = channels of input row (base+j-1)  ("h0"),
              partitions 64:128 = channels of input row (base+j)  ("h1"),
              so a 128-partition column at slot j carries TWO adjacent rows.
  output      [9, 176, 1216] f16  (9 planes; plane 4 holds -R*T, host adds 1)

Conv restructure: for output row r, taps di=0,1 contract in ONE 128-partition
matmul (slots j=r..r+1 read rows r-1,r in h0/h1), tap di=2 in a 64-partition
matmul (h0 slot r+2 for bank-A groups, h1 slot r+1 for bank-B groups; the two
64-part tiles co-issue on opposite PE row halves). 6 matmul rows/pixel vs 9.

PSUM packing: 4 row-pair groups per bank at column tile positions 32g, so one
ACT instruction evacuates a whole bank (bias from b9x; garbage partitions get
bias 0 on top of once-zeroed PSUM => exact zeros, no per-chunk memsets).
abs on Pool (tensor_tensor abs_max), reciprocal on ACT (table), selector
matmuls (S-reduce + R-broadcast) as in v1. Output staged/DMA'd in f16.
"""

import sys

if "/opt/trn_rl_repo" not in sys.path:
    sys.path.insert(0, "/opt/trn_rl_repo")

import numpy as np
import concourse.bass as bass
import concourse.mybir as mybir
from concourse.tile import TileContext
from concourse.vector_clock import ScopedClock, VectorClock


# ---- toolchain workarounds (drain-wait split, per-instruction sync-wait
# limit, optional NTFF profiling shim) ----
def _drain_and_barrier_split(self, tick_clock, wait_clock):
    gclock = tick_clock.global_clock
    nprocs = len(gclock)
    # One NOP per nonzero proc tick; add_sem_waits elides already-observed
    # ticks, so each NOP carries at most one wait.
    for proc in range(nprocs):
        tick = gclock[proc]
        if tick <= 0:
            continue
        vc = VectorClock([0] * nprocs)
        vc.require_at_least(proc, tick)
        nop = self.nc.sync.nop(nofuse=True, hint="drain_split_wait")
        wait_clock.add_sem_waits(nop.ins, ScopedClock({None: vc}))

    # All waits were attached to the NOPs above (same engine, program order),
    # so the drain itself needs none — keeping it under the CoreV3 codegen
    # limit on sync-wait commands for Drain.
    self.nc.sync.drain()

    self.nc.all_engine_barrier()
    assert self.sems is not None
    popped = self.nc._tile_sem_poison_stack.pop()
    assert popped is self._sem_poison
    self.nc.clear_and_free_semaphores(list(self.sems.allocated().values()))
    self.nc.all_engine_barrier()


def install():
    TileContext._drain_and_barrier = _drain_and_barrier_split
    install_wait_split()


_MAX_WAITS = 1


def _split_waits_json(bir: bytes) -> bytes:
    """Walrus in this toolchain rejects instructions carrying more than one
    sync-wait command ("Too many sync wait commands"). Move excess waits onto
    same-engine NoOps inserted immediately before the instruction."""
    import orjson

    m = orjson.loads(bir)
    for func in m.get("functions", []):
        for block in func.get("blocks", []):
            out = []
            changed = False
            for inst in block["instructions"]:
                si = inst.get("sync_info") or {}
                waits = si.get("on_wait") or []
                if len(waits) > _MAX_WAITS:
                    keep = waits[-_MAX_WAITS:]
                    extra = waits[:-_MAX_WAITS]
                    for k, w in enumerate(extra):
                        out.append(
                            {
                                "debug": inst.get("debug", 0),
                                "engine": inst["engine"],
                                "ins": [],
                                "name": f"{inst['name']}-wsplit{k}",
                                "opcode": "NoOp",
                                "outs": [],
                                "sync_info": {"on_update": [], "on_wait": [w]},
                                "text_hint": "wait_split",
                            }
                        )
                    si["on_wait"] = keep
                    inst["sync_info"] = si
                    changed = True
                out.append(inst)
            if changed:
                block["instructions"] = out
    return orjson.dumps(m)


def install_wait_split():
    import concourse.bass as _bass

    if getattr(_bass.Bass, "_wait_split_installed", False):
        return
    orig = _bass.Bass.to_json_bytes

    def to_json_bytes(self):
        return _split_waits_json(orig(self))

    _bass.Bass.to_json_bytes = to_json_bytes
    _bass.Bass._wait_split_installed = True


def install_ntff_shim():
    """Provide the missing ``antenv.axon_hooks`` module so trace=True works
    under axon, wiring it to trn_boot's ctypes NTFF hook factory."""
    import sys
    import types

    if "antenv.axon_hooks" in sys.modules:
        return
    mod = types.ModuleType("antenv.axon_hooks")
    state = {"hook": None}

    def set_axon_ntff_profile_hook(h):
        state["hook"] = h

    def get_axon_ntff_profile_hook():
        return state["hook"]

    mod.set_axon_ntff_profile_hook = set_axon_ntff_profile_hook
    mod.get_axon_ntff_profile_hook = get_axon_ntff_profile_hook
    sys.modules["antenv.axon_hooks"] = mod

    try:
        from trn_agent_boot.trn_boot import _ntff_profile_via_ctypes

        hook = _ntff_profile_via_ctypes("/opt/axon/libaxon_pjrt.so")
        if hook is not None:
            set_axon_ntff_profile_hook(hook)
    except Exception as e:  # profiling optional — degrade to no trace
        print(f"ntff shim: hook install failed: {e}")

    # upload_artifacts pushes to a remote bucket that doesn't exist in this
    # container; stub it so trace post-processing stays local.
    from concourse import bass_utils

    bass_utils.upload_artifacts = lambda tmpdir: tmpdir


# geometry
B, C, H, W, K = 4, 64, 352, 1216, 3
OC = 8
HALF = 176  # rows per core
WP = W + 2  # padded width
TR = 16  # output rows per block
SLOT = TR + 2  # row-slots per block slab
NBLK = HALF // TR  # 11
XCH = [(0, 256), (256, 256), (512, 256), (768, 256), (1024, 192)]
MROW = [0, 1, 2, 3, 8, 4, 5, 6, 7]  # kernel row m for output plane p

F32 = mybir.dt.float32
F32R = mybir.dt.float32r
F16 = mybir.dt.float16
AF = mybir.ActivationFunctionType


def build_nc():
    nc = bass.Bass()
    slab = nc.dram_tensor("slab", [128, NBLK, SLOT * WP], F16, kind="ExternalInput")
    w6 = nc.dram_tensor("w6", [128, 54], F16, kind="ExternalInput")
    b9x = nc.dram_tensor("b9x", [128, 1], F32, kind="ExternalInput")
    ssel = nc.dram_tensor("ssel", [128, 18], F16, kind="ExternalInput")
    bsel = nc.dram_tensor("bsel", [128, 1024], F32, kind="ExternalInput")
    out = nc.dram_tensor("out", [9, HALF, W], F16, kind="ExternalOutput")

    with TileContext(nc) as tc:
        with (
            tc.tile_pool(name="consts", bufs=1) as cpool,
            tc.tile_pool(name="slabp", bufs=3) as slabp,
            tc.tile_pool(name="work", bufs=5) as work,
            tc.tile_pool(name="stagep", bufs=2) as stagep,
            tc.tile_pool(name="pcA", bufs=2, space="PSUM") as pcA,
            tc.tile_pool(name="pcB", bufs=2, space="PSUM") as pcB,
            tc.tile_pool(name="ps_s8", bufs=2, space="PSUM") as ps_s8,
            tc.tile_pool(name="ps_rbc", bufs=1, space="PSUM") as ps_rbc,
        ):
            w6t = cpool.tile([128, 54], F16, name="w6t")
            b9xt = cpool.tile([128, 1], F32, name="b9xt")
            sselt = cpool.tile([128, 18], F16, name="sselt")
            bselt = cpool.tile([128, 1024], F32R, name="bselt")
            nc.gpsimd.dma_start(w6t[:], w6[:])
            nc.gpsimd.dma_start(b9xt[:], b9x[:])
            nc.gpsimd.dma_start(sselt[:], ssel[:])
            nc.gpsimd.dma_start(bselt[:], bsel[:])

            # One-time zero of the conv PSUM banks: matmuls only ever write
            # partitions 32g..32g+9, the evacuation reads all 128, and the
            # garbage partitions must be exactly 0.0 (bias b9x is 0 there).
            for pool, nm in ((pcA, "cA"), (pcB, "cB")):
                for _ in range(2):
                    zt = pool.tile([128, 512], F32, name=nm)
                    nc.vector.memset(zt[:], 0.0)
            # S banks start at 1.0 so unused partitions reciprocate to a
            # finite 1.0 (they hit zero bsel weights; 0*inf would be NaN)
            for _ in range(2):
                zs = ps_s8.tile([128, 512], F32, name="s8")
                nc.vector.memset(zs[:], 1.0)

            # Software pipeline (as v1): each chunk's selector matmuls are
            # deferred behind the NEXT chunk's conv (stage 1: S-matmul +
            # reciprocal) and 4 chunks later (stage 2: broadcast matmul +
            # multiplies + output DMAs) to keep the in-order PE queue dense.
            from collections import deque

            pending = deque()

            # S-sums of 4 consecutive chunks (phases) share one PSUM bank at
            # partition offsets 32*ph, so ONE [128, n] DVE reciprocal serves
            # 4 chunks (the reciprocal's cost depends on columns, not rows).
            group_state = {}

            def emit_recip():
                s8x = group_state.pop("s8x", None)
                if s8x is None:
                    return
                r8x = work.tile([128, 512], F32R, name="r8")
                with nc.allow_low_precision(reason="f32r normalize"):
                    nc.vector.reciprocal(r8x[:, 0:256], s8x[:, 0:256])
                    nc.vector.reciprocal(r8x[:, 256:512], s8x[:, 256:512])
                for stt_i in pending:
                    if "r8x" not in stt_i:
                        stt_i["r8x"] = r8x

            def stage1(stt):
                n = stt["n"]
                ph = stt["cno"] % 4
                stt["ph"] = ph
                if ph == 0:
                    group_state["s8x"] = ps_s8.tile([128, 512], F32, name="s8")
                s8x = group_state["s8x"]
                for h in range(2):
                    nc.tensor.matmul(
                        out=s8x[32 * ph : 32 * ph + 9, 0:n],
                        lhsT=sselt[:, 9 * h : 9 * h + 9],
                        rhs=stt["aas"][h][:, 0:n],
                        start=(h == 0),
                        stop=(h == 1),
                        tile_position=(0, 32 * ph),
                    )
                if ph == 3:
                    emit_recip()

            def stage2(stt):
                n, w, x0, ph = stt["n"], stt["w"], stt["x0"], stt["ph"]
                rbcs = [
                    ps_rbc.tile([128, 512], F32, name="rbcA"),
                    ps_rbc.tile([128, 512], F32, name="rbcB"),
                ]
                for h in range(2):
                    nc.tensor.matmul(
                        out=rbcs[h][:, 0:n],
                        lhsT=bselt[:, 256 * ph + 128 * h : 256 * ph + 128 * h + 128],
                        rhs=stt["r8x"][:, 0:n],
                        start=True,
                        stop=True,
                        tile_position=(0, 0),
                    )
                # stage per-partition layout (h, r, x): 2 x 2 x W
                sv = stt["stage"][:].rearrange("p (h r x) -> p h r x", h=2, r=2, x=W)
                for h in range(2):
                    nc.vector.tensor_mul(
                        sv[:, h, :, x0 : x0 + w],
                        stt["ys"][h][:, 0:n].rearrange("p (r x) -> p r x", r=2, x=w),
                        rbcs[h][:, 0:n].rearrange("p (r x) -> p r x", r=2, x=w),
                    )
                if stt["last_chunk"]:
                    blk = stt["blk"]
                    for p in range(9):
                        m = MROW[p]
                        sb = stt["stage"][:].rearrange(
                            "(g m) (h r x) -> g m h r x", g=4, m=32, h=2, r=2, x=W
                        )[:, m, :, :, :]
                        dr = out[p].rearrange(
                            "(a h g r) w -> a g h r w", a=NBLK, h=2, g=4, r=2
                        )[blk]
                        nc.sync.dma_start(dr, sb)

            def advance(newstate):
                if len(pending) >= 1:
                    stage1(pending[-1])
                if len(pending) >= 4:
                    stage2(pending.popleft())
                if newstate is not None:
                    pending.append(newstate)

            st = slabp.tile([128, SLOT * WP], F16, name="st")
            nc.gpsimd.dma_start(st[:], slab[:, 0, :])
            st_next = slabp.tile([128, SLOT * WP], F16, name="st")
            nc.gpsimd.dma_start(st_next[:], slab[:, 1, :])
            for blk in range(NBLK):
                st_next2 = None
                # 2D view: [128, SLOT, WP]
                stv = st[:].rearrange("p (r w) -> p r w", r=SLOT, w=WP)
                stage = stagep.tile([128, 4 * W], F16, name="stageAB")
                for ci, (x0, w) in enumerate(XCH):
                    n = 2 * w  # elems per row-pair (2 rows of w)
                    cA = pcA.tile([128, 512], F32, name="cA")
                    cB = pcB.tile([128, 512], F32, name="cB")
                    banks = (cA, cB)
                    # pass 1: taps di=0,1 via one 128-part matmul per pair
                    for dj in range(3):
                        for g8 in range(8):
                            h, g = divmod(g8, 4)
                            j = 8 * h + 2 * g
                            cv = banks[h][32 * g : 32 * g + 9, 0:n].rearrange(
                                "p (r x) -> p r x", r=2, x=w
                            )
                            nc.tensor.matmul(
                                out=cv,
                                lhsT=w6t[:, 9 * dj : 9 * dj + 9],
                                rhs=stv[:, j : j + 2, x0 + dj : x0 + dj + w],
                                start=(dj == 0),
                                stop=False,
                                tile_position=(0, 32 * g),
                            )
                    # pass 2: tap di=2 via 64-part matmuls; bank A reads h0
                    # (slots j+2..j+3), bank B reads h1 (slots j+1..j+2) so
                    # adjacent instructions co-issue on opposite PE halves
                    # and write different PSUM banks.
                    for dj in range(3):
                        for g in range(4):
                            for h in range(2):
                                j = 8 * h + 2 * g
                                cv = banks[h][32 * g : 32 * g + 9, 0:n].rearrange(
                                    "p (r x) -> p r x", r=2, x=w
                                )
                                if h == 0:
                                    rhs = stv[0:64, j + 2 : j + 4, x0 + dj : x0 + dj + w]
                                    lhsT = w6t[0:64, 27 + 9 * dj : 36 + 9 * dj]
                                    tp = (0, 32 * g)
                                else:
                                    rhs = stv[
                                        64:128, j + 1 : j + 3, x0 + dj : x0 + dj + w
                                    ]
                                    lhsT = w6t[64:128, 27 + 9 * dj : 36 + 9 * dj]
                                    tp = (64, 32 * g)
                                nc.tensor.matmul(
                                    out=cv,
                                    lhsT=lhsT,
                                    rhs=rhs,
                                    start=False,
                                    stop=(dj == 2),
                                    tile_position=tp,
                                )

                    newstate = {
                        "n": n,
                        "w": w,
                        "x0": x0,
                        "stage": stage,
                        "last_chunk": ci == len(XCH) - 1,
                        "blk": blk,
                        "cno": blk * len(XCH) + ci,
                    }
                    ys = [
                        work.tile([128, 512], F32, name="yA"),
                        work.tile([128, 512], F32, name="yB"),
                    ]
                    aas = [
                        work.tile([128, 512], F16, name="aA"),
                        work.tile([128, 512], F16, name="aB"),
                    ]
                    newstate["ys"] = ys
                    newstate["aas"] = aas
                    advance(newstate)

                    if ci == 0 and blk + 2 < NBLK:
                        st_next2 = slabp.tile([128, SLOT * WP], F16, name="st")
                        nc.gpsimd.dma_start(st_next2[:], slab[:, blk + 2, :])

                    # evacuate whole banks: y = psum + bias (garbage rows 0),
                    # then |y| on Pool for the S-selector contraction
                    for h in range(2):
                        nc.scalar.activation(
                            ys[h][:, 0:n],
                            banks[h][:, 0:n],
                            AF.Identity,
                            bias=b9xt[:, 0:1],
                            scale=1.0,
                        )
                    for h in range(2):
                        nc.scalar.activation(
                            aas[h][:, 0:n],
                            banks[h][:, 0:n],
                            AF.Abs,
                            bias=b9xt[:, 0:1],
                            scale=1.0,
                        )
                st, st_next = st_next, st_next2
            # drain the pipeline
            if pending:
                stage1(pending[-1])
            emit_recip()  # reciprocal for a trailing partial phase-group
            while pending:
                stage2(pending.popleft())
    return nc


def make_consts(conv_w, gamma, beta, mean, var):
    eps = 1e-5
    s = gamma.astype(np.float64) / np.sqrt(var.astype(np.float64) + eps)
    bt = beta.astype(np.float64) - mean.astype(np.float64) * s
    wp = conv_w.astype(np.float64) * s[:, None, None, None]  # [8, 64, 3, 3]
    w9 = np.concatenate([wp, wp.sum(axis=0, keepdims=True)])  # [9, 64, 3, 3]

    w6 = np.zeros((128, 54), np.float16)
    for dj in range(3):
        # pass1: partitions 0:64 multiply tap di=0, 64:128 tap di=1
        w6[0:64, 9 * dj : 9 * dj + 9] = w9[:, :, 0, dj].T.astype(np.float16)
        w6[64:128, 9 * dj : 9 * dj + 9] = w9[:, :, 1, dj].T.astype(np.float16)
        # pass2: tap di=2 weights replicated in both halves
        w6[0:64, 27 + 9 * dj : 36 + 9 * dj] = w9[:, :, 2, dj].T.astype(np.float16)
        w6[64:128, 27 + 9 * dj : 36 + 9 * dj] = w9[:, :, 2, dj].T.astype(np.float16)

    bt9 = np.concatenate([bt, [bt.sum()]]).astype(np.float32)  # [9]
    b9x = np.zeros((128, 1), np.float32)
    for g in range(4):
        b9x[32 * g : 32 * g + 9, 0] = bt9

    # ssel cols 0:9 (bank A): group-g channel rows -> S row g; cols 9:18
    # (bank B): -> S row 4+g. Col 8 is a dummy 9th output (the dst-partition
    # ISA check wants the same 9-wide shape as the conv matmuls); it gets a
    # copy of S row 0 so its reciprocal stays finite.
    ssel = np.zeros((128, 18), np.float16)
    for g in range(4):
        ssel[32 * g : 32 * g + 8, g] = 1.0
        ssel[32 * g : 32 * g + 8, 9 + 4 + g] = 1.0
    ssel[0:8, 8] = 1.0
    # bsel, per phase ph (column block 256*ph), contracting all 128 R rows:
    # R row 32ph+g -> rows 32g+c (+1 for c<8, -1 for c=8) for bank A (h=0,
    # cols +0:128) and R row 32ph+4+g likewise for bank B (cols +128:256).
    # All other rows have zero weight.
    bsel = np.zeros((128, 1024), np.float32)
    for ph in range(4):
        for g in range(4):
            bsel[32 * ph + g, 256 * ph + 32 * g : 256 * ph + 32 * g + 8] = 1.0
            bsel[32 * ph + g, 256 * ph + 32 * g + 8] = -1.0
            bsel[32 * ph + 4 + g, 256 * ph + 128 + 32 * g : 256 * ph + 128 + 32 * g + 8] = 1.0
            bsel[32 * ph + 4 + g, 256 * ph + 128 + 32 * g + 8] = -1.0
    return w6, b9x, ssel, bsel


TRACE = False
LAST_EXEC_NS = None


def kernel(feature, conv_w, gamma, beta, mean, var, kernel_size):
    global LAST_EXEC_NS
    install()
    if TRACE:
        install_ntff_shim()

    from concourse.bass_utils import run_bass_kernel_spmd

    feature = np.asarray(feature, np.float32)
    conv_w = np.asarray(conv_w, np.float32)
    gamma = np.asarray(gamma, np.float32)
    beta = np.asarray(beta, np.float32)
    mean = np.asarray(mean, np.float32)
    var = np.asarray(var, np.float32)

    w6, b9x, ssel, bsel = make_consts(conv_w, gamma, beta, mean, var)

    # padded feature [B, C, H+3, W+2] in f16 (one extra zero row at the
    # bottom so the unused h1 slot 17 of the last block stays in range)
    fpad = np.zeros((B, C, H + 3, WP), np.float16)
    fpad[:, :, 1 : H + 1, 1 : W + 1] = feature

    in_maps = []
    for core in range(8):
        b, half = core // 2, core % 2
        h0 = half * HALF
        slab2 = np.empty((128, NBLK, SLOT * WP), np.float16)
        for k in range(NBLK):
            base = h0 + TR * k  # fpad row of slot 0 for the h0 half
            slab2[0:64, k, :] = fpad[b, :, base : base + SLOT, :].reshape(C, -1)
            slab2[64:128, k, :] = fpad[b, :, base + 1 : base + 1 + SLOT, :].reshape(
                C, -1
            )
        in_maps.append(
            {
                "slab": slab2,
                "w6": w6,
                "b9x": b9x,
                "ssel": ssel,
                "bsel": bsel,
            }
        )

    nc = build_nc()
    res = run_bass_kernel_spmd(nc, in_maps, core_ids=list(range(8)), trace=TRACE)
    LAST_EXEC_NS = res.exec_time_ns

    out_full = np.zeros((B, 9, H + 2, WP), np.float32)
    for core in range(8):
        b, half = core // 2, core % 2
        h0 = half * HALF
        r = np.asarray(res.results[core]["out"], dtype=np.float32)  # [9, 176, 1216]
        for p in range(9):
            i, j = p // 3, p % 3
            plane = r[p]
            if p == 4:
                plane = 1.0 + plane
            out_full[b, p, h0 + i : h0 + HALF + i, j : j + W] = plane
    return out_full


# revision 54
# speedup vs baseline: 1.4048x; 1.1584x over previous
"""CSPNGenerate Trainium2 kernel (v2: f16 row-duplicated slab).

Per core (8 cores = batch b in 0..3  x  half in 0..1):
  input slab  [128, 11, 18*1218] f16: per 16-row block, 18 row-slots;
              partitions 0:64 = channels of input row (base+j-1)  ("h0"),
              partitions 64:128 = channels of input row (base+j)  ("h1"),
              so a 128-partition column at slot j carries TWO adjacent rows.
  output      [9, 176, 1216] f16  (9 planes; plane 4 holds -R*T, host adds 1)

Conv restructure: for output row r, taps di=0,1 contract in ONE 128-partition
matmul (slots j=r..r+1 read rows r-1,r in h0/h1), tap di=2 in a 64-partition
matmul (h0 slot r+2 for bank-A groups, h1 slot r+1 for bank-B groups; the two
64-part tiles co-issue on opposite PE row halves). 6 matmul rows/pixel vs 9.

PSUM packing: 4 row-pair groups per bank at column tile positions 32g, so one
ACT instruction evacuates a whole bank (bias from b9x; garbage partitions get
bias 0 on top of once-zeroed PSUM => exact zeros, no per-chunk memsets).
abs on Pool (tensor_tensor abs_max), reciprocal on ACT (table), selector
matmuls (S-reduce + R-broadcast) as in v1. Output staged/DMA'd in f16.
"""

import sys

if "/opt/trn_rl_repo" not in sys.path:
    sys.path.insert(0, "/opt/trn_rl_repo")

import numpy as np
import concourse.bass as bass
import concourse.mybir as mybir
from concourse.tile import TileContext
from concourse.vector_clock import ScopedClock, VectorClock


# ---- toolchain workarounds (drain-wait split, per-instruction sync-wait
# limit, optional NTFF profiling shim) ----
def _drain_and_barrier_split(self, tick_clock, wait_clock):
    gclock = tick_clock.global_clock
    nprocs = len(gclock)
    # One NOP per nonzero proc tick; add_sem_waits elides already-observed
    # ticks, so each NOP carries at most one wait.
    for proc in range(nprocs):
        tick = gclock[proc]
        if tick <= 0:
            continue
        vc = VectorClock([0] * nprocs)
        vc.require_at_least(proc, tick)
        nop = self.nc.sync.nop(nofuse=True, hint="drain_split_wait")
        wait_clock.add_sem_waits(nop.ins, ScopedClock({None: vc}))

    # All waits were attached to the NOPs above (same engine, program order),
    # so the drain itself needs none — keeping it under the CoreV3 codegen
    # limit on sync-wait commands for Drain.
    self.nc.sync.drain()

    self.nc.all_engine_barrier()
    assert self.sems is not None
    popped = self.nc._tile_sem_poison_stack.pop()
    assert popped is self._sem_poison
    self.nc.clear_and_free_semaphores(list(self.sems.allocated().values()))
    self.nc.all_engine_barrier()


def install():
    TileContext._drain_and_barrier = _drain_and_barrier_split
    install_wait_split()


_MAX_WAITS = 1


def _split_waits_json(bir: bytes) -> bytes:
    """Walrus in this toolchain rejects instructions carrying more than one
    sync-wait command ("Too many sync wait commands"). Move excess waits onto
    same-engine NoOps inserted immediately before the instruction."""
    import orjson

    m = orjson.loads(bir)
    for func in m.get("functions", []):
        for block in func.get("blocks", []):
            out = []
            changed = False
            for inst in block["instructions"]:
                si = inst.get("sync_info") or {}
                waits = si.get("on_wait") or []
                if len(waits) > _MAX_WAITS:
                    keep = waits[-_MAX_WAITS:]
                    extra = waits[:-_MAX_WAITS]
                    for k, w in enumerate(extra):
                        out.append(
                            {
                                "debug": inst.get("debug", 0),
                                "engine": inst["engine"],
                                "ins": [],
                                "name": f"{inst['name']}-wsplit{k}",
                                "opcode": "NoOp",
                                "outs": [],
                                "sync_info": {"on_update": [], "on_wait": [w]},
                                "text_hint": "wait_split",
                            }
                        )
                    si["on_wait"] = keep
                    inst["sync_info"] = si
                    changed = True
                out.append(inst)
            if changed:
                block["instructions"] = out
    return orjson.dumps(m)


def install_wait_split():
    import concourse.bass as _bass

    if getattr(_bass.Bass, "_wait_split_installed", False):
        return
    orig = _bass.Bass.to_json_bytes

    def to_json_bytes(self):
        return _split_waits_json(orig(self))

    _bass.Bass.to_json_bytes = to_json_bytes
    _bass.Bass._wait_split_installed = True


def install_ntff_shim():
    """Provide the missing ``antenv.axon_hooks`` module so trace=True works
    under axon, wiring it to trn_boot's ctypes NTFF hook factory."""
    import sys
    import types

    if "antenv.axon_hooks" in sys.modules:
        return
    mod = types.ModuleType("antenv.axon_hooks")
    state = {"hook": None}

    def set_axon_ntff_profile_hook(h):
        state["hook"] = h

    def get_axon_ntff_profile_hook():
        return state["hook"]

    mod.set_axon_ntff_profile_hook = set_axon_ntff_profile_hook
    mod.get_axon_ntff_profile_hook = get_axon_ntff_profile_hook
    sys.modules["antenv.axon_hooks"] = mod

    try:
        from trn_agent_boot.trn_boot import _ntff_profile_via_ctypes

        hook = _ntff_profile_via_ctypes("/opt/axon/libaxon_pjrt.so")
        if hook is not None:
            set_axon_ntff_profile_hook(hook)
    except Exception as e:  # profiling optional — degrade to no trace
        print(f"ntff shim: hook install failed: {e}")

    # upload_artifacts pushes to a remote bucket that doesn't exist in this
    # container; stub it so trace post-processing stays local.
    from concourse import bass_utils

    bass_utils.upload_artifacts = lambda tmpdir: tmpdir


# geometry
B, C, H, W, K = 4, 64, 352, 1216, 3
OC = 8
HALF = 176  # rows per core
WP = W + 2  # padded width
TR = 16  # output rows per block
SLOT = TR + 2  # row-slots per block slab
NBLK = HALF // TR  # 11
XCH = [(0, 256), (256, 256), (512, 256), (768, 256), (1024, 192)]
MROW = [0, 1, 2, 3, 8, 4, 5, 6, 7]  # kernel row m for output plane p

F32 = mybir.dt.float32
F32R = mybir.dt.float32r
F16 = mybir.dt.float16
AF = mybir.ActivationFunctionType


def build_nc():
    nc = bass.Bass()
    slab = nc.dram_tensor("slab", [128, NBLK, SLOT * WP], F16, kind="ExternalInput")
    w6 = nc.dram_tensor("w6", [128, 54], F16, kind="ExternalInput")
    b9x = nc.dram_tensor("b9x", [128, 1], F32, kind="ExternalInput")
    ssel = nc.dram_tensor("ssel", [128, 18], F16, kind="ExternalInput")
    bsel = nc.dram_tensor("bsel", [128, 1024], F32, kind="ExternalInput")
    out = nc.dram_tensor("out", [9, HALF, W], F16, kind="ExternalOutput")

    with TileContext(nc) as tc:
        with (
            tc.tile_pool(name="consts", bufs=1) as cpool,
            tc.tile_pool(name="slabp", bufs=2) as slabp,
            tc.tile_pool(name="work", bufs=7) as work,
            tc.tile_pool(name="stagep", bufs=3) as stagep,
            tc.tile_pool(name="pcA", bufs=2, space="PSUM") as pcA,
            tc.tile_pool(name="pcB", bufs=2, space="PSUM") as pcB,
            tc.tile_pool(name="ps_s8", bufs=2, space="PSUM") as ps_s8,
            tc.tile_pool(name="ps_rbc", bufs=1, space="PSUM") as ps_rbc,
        ):
            w6t = cpool.tile([128, 54], F16, name="w6t")
            b9xt = cpool.tile([128, 1], F32, name="b9xt")
            sselt = cpool.tile([128, 18], F16, name="sselt")
            bselt = cpool.tile([128, 1024], F32R, name="bselt")
            nc.gpsimd.dma_start(w6t[:], w6[:])
            nc.gpsimd.dma_start(b9xt[:], b9x[:])
            nc.gpsimd.dma_start(sselt[:], ssel[:])
            nc.gpsimd.dma_start(bselt[:], bsel[:])

            # One-time zero of the conv PSUM banks: matmuls only ever write
            # partitions 32g..32g+9, the evacuation reads all 128, and the
            # garbage partitions must be exactly 0.0 (bias b9x is 0 there).
            for pool, nm in ((pcA, "cA"), (pcB, "cB")):
                for _ in range(2):
                    zt = pool.tile([128, 512], F32, name=nm)
                    nc.vector.memset(zt[:], 0.0)
            # S banks start at 1.0 so unused partitions reciprocate to a
            # finite 1.0 (they hit zero bsel weights; 0*inf would be NaN)
            for _ in range(2):
                zs = ps_s8.tile([128, 512], F32, name="s8")
                nc.vector.memset(zs[:], 1.0)

            # Software pipeline (as v1): each chunk's selector matmuls are
            # deferred behind the NEXT chunk's conv (stage 1: S-matmul +
            # reciprocal) and 4 chunks later (stage 2: broadcast matmul +
            # multiplies + output DMAs) to keep the in-order PE queue dense.
            from collections import deque

            pending = deque()

            # S-sums of 4 consecutive chunks (phases) share one PSUM bank at
            # partition offsets 32*ph, so ONE [128, n] DVE reciprocal serves
            # 4 chunks (the reciprocal's cost depends on columns, not rows).
            group_state = {}

            def emit_recip():
                s8x = group_state.pop("s8x", None)
                if s8x is None:
                    return
                r8x = work.tile([128, 512], F32R, name="r8")
                with nc.allow_low_precision(reason="f32r normalize"):
                    nc.vector.reciprocal(r8x[:, 0:256], s8x[:, 0:256])
                    nc.vector.reciprocal(r8x[:, 256:512], s8x[:, 256:512])
                for stt_i in pending:
                    if "r8x" not in stt_i:
                        stt_i["r8x"] = r8x

            def stage1(stt):
                n = stt["n"]
                ph = stt["cno"] % 4
                stt["ph"] = ph
                if ph == 0:
                    group_state["s8x"] = ps_s8.tile([128, 512], F32, name="s8")
                s8x = group_state["s8x"]
                for h in range(2):
                    nc.tensor.matmul(
                        out=s8x[32 * ph : 32 * ph + 9, 0:n],
                        lhsT=sselt[:, 9 * h : 9 * h + 9],
                        rhs=stt["aas"][h][:, 0:n],
                        start=(h == 0),
                        stop=(h == 1),
                        tile_position=(0, 32 * ph),
                    )
                if ph == 3:
                    emit_recip()

            def stage2(stt):
                n, w, x0, ph = stt["n"], stt["w"], stt["x0"], stt["ph"]
                rbcs = [
                    ps_rbc.tile([128, 512], F32, name="rbcA"),
                    ps_rbc.tile([128, 512], F32, name="rbcB"),
                ]
                for h in range(2):
                    nc.tensor.matmul(
                        out=rbcs[h][:, 0:n],
                        lhsT=bselt[:, 256 * ph + 128 * h : 256 * ph + 128 * h + 128],
                        rhs=stt["r8x"][:, 0:n],
                        start=True,
                        stop=True,
                        tile_position=(0, 0),
                    )
                # stage per-partition layout (h, r, x): 2 x 2 x W
                sv = stt["stage"][:].rearrange("p (h r x) -> p h r x", h=2, r=2, x=W)
                for h in range(2):
                    nc.vector.tensor_mul(
                        sv[:, h, :, x0 : x0 + w],
                        stt["ys"][h][:, 0:n].rearrange("p (r x) -> p r x", r=2, x=w),
                        rbcs[h][:, 0:n].rearrange("p (r x) -> p r x", r=2, x=w),
                    )
                if stt["last_chunk"]:
                    blk = stt["blk"]
                    for p in range(9):
                        m = MROW[p]
                        sb = stt["stage"][:].rearrange(
                            "(g m) (h r x) -> g m h r x", g=4, m=32, h=2, r=2, x=W
                        )[:, m, :, :, :]
                        dr = out[p].rearrange(
                            "(a h g r) w -> a g h r w", a=NBLK, h=2, g=4, r=2
                        )[blk]
                        nc.sync.dma_start(dr, sb)

            def advance(newstate):
                if len(pending) >= 1:
                    stage1(pending[-1])
                # lag 6 (not 4) so the phase-group reciprocal on DVE has ~2
                # chunks of PE work to hide behind before rbc consumes it
                if len(pending) >= 6:
                    stage2(pending.popleft())
                if newstate is not None:
                    pending.append(newstate)

            # block 0 loaded in two column ranges so the first chunk's conv
            # (cols 0:259) can start ~4x sooner than a full-block load
            st = slabp.tile([128, SLOT * WP], F16, name="st")
            st0v = st[:].rearrange("p (r w) -> p r w", r=SLOT, w=WP)
            sl0v = slab[:, 0, :].rearrange("p (r w) -> p r w", r=SLOT, w=WP)
            nc.gpsimd.dma_start(st0v[:, :, 0:272], sl0v[:, :, 0:272])
            nc.gpsimd.dma_start(st0v[:, :, 272:WP], sl0v[:, :, 272:WP])
            for blk in range(NBLK):
                st_next2 = None
                # 2D view: [128, SLOT, WP]
                stv = st[:].rearrange("p (r w) -> p r w", r=SLOT, w=WP)
                stage = stagep.tile([128, 4 * W], F16, name="stageAB")
                for ci, (x0, w) in enumerate(XCH):
                    n = 2 * w  # elems per row-pair (2 rows of w)
                    cA = pcA.tile([128, 512], F32, name="cA")
                    cB = pcB.tile([128, 512], F32, name="cB")
                    banks = (cA, cB)
                    # pass 1: taps di=0,1 via one 128-part matmul per pair
                    for dj in range(3):
                        for g8 in range(8):
                            h, g = divmod(g8, 4)
                            j = 8 * h + 2 * g
                            cv = banks[h][32 * g : 32 * g + 9, 0:n].rearrange(
                                "p (r x) -> p r x", r=2, x=w
                            )
                            nc.tensor.matmul(
                                out=cv,
                                lhsT=w6t[:, 9 * dj : 9 * dj + 9],
                                rhs=stv[:, j : j + 2, x0 + dj : x0 + dj + w],
                                start=(dj == 0),
                                stop=False,
                                tile_position=(0, 32 * g),
                            )
                    # pass 2: tap di=2 via 64-part matmuls; bank A reads h0
                    # (slots j+2..j+3), bank B reads h1 (slots j+1..j+2) so
                    # adjacent instructions co-issue on opposite PE halves
                    # and write different PSUM banks.
                    for dj in range(3):
                        for g in range(4):
                            for h in range(2):
                                j = 8 * h + 2 * g
                                cv = banks[h][32 * g : 32 * g + 9, 0:n].rearrange(
                                    "p (r x) -> p r x", r=2, x=w
                                )
                                if h == 0:
                                    rhs = stv[0:64, j + 2 : j + 4, x0 + dj : x0 + dj + w]
                                    lhsT = w6t[0:64, 27 + 9 * dj : 36 + 9 * dj]
                                    tp = (0, 32 * g)
                                else:
                                    rhs = stv[
                                        64:128, j + 1 : j + 3, x0 + dj : x0 + dj + w
                                    ]
                                    lhsT = w6t[64:128, 27 + 9 * dj : 36 + 9 * dj]
                                    tp = (64, 32 * g)
                                nc.tensor.matmul(
                                    out=cv,
                                    lhsT=lhsT,
                                    rhs=rhs,
                                    start=False,
                                    stop=(dj == 2),
                                    tile_position=tp,
                                )

                    newstate = {
                        "n": n,
                        "w": w,
                        "x0": x0,
                        "stage": stage,
                        "last_chunk": ci == len(XCH) - 1,
                        "blk": blk,
                        "cno": blk * len(XCH) + ci,
                    }
                    ys = [
                        work.tile([128, 512], F32, name="yA"),
                        work.tile([128, 512], F32, name="yB"),
                    ]
                    aas = [
                        work.tile([128, 512], F16, name="aA"),
                        work.tile([128, 512], F16, name="aB"),
                    ]
                    newstate["ys"] = ys
                    newstate["aas"] = aas
                    advance(newstate)

                    if ci == 0 and blk + 1 < NBLK:
                        st_next2 = slabp.tile([128, SLOT * WP], F16, name="st")
                        nc.gpsimd.dma_start(st_next2[:], slab[:, blk + 1, :])

                    # evacuate whole banks: y = psum + bias (garbage rows 0),
                    # then |y| on Pool for the S-selector contraction
                    for h in range(2):
                        nc.scalar.activation(
                            ys[h][:, 0:n],
                            banks[h][:, 0:n],
                            AF.Identity,
                            bias=b9xt[:, 0:1],
                            scale=1.0,
                        )
                    for h in range(2):
                        nc.scalar.activation(
                            aas[h][:, 0:n],
                            banks[h][:, 0:n],
                            AF.Abs,
                            bias=b9xt[:, 0:1],
                            scale=1.0,
                        )
                st = st_next2
            # drain the pipeline
            if pending:
                stage1(pending[-1])
            emit_recip()  # reciprocal for a trailing partial phase-group
            while pending:
                stage2(pending.popleft())
    return nc


def make_consts(conv_w, gamma, beta, mean, var):
    eps = 1e-5
    s = gamma.astype(np.float64) / np.sqrt(var.astype(np.float64) + eps)
    bt = beta.astype(np.float64) - mean.astype(np.float64) * s
    wp = conv_w.astype(np.float64) * s[:, None, None, None]  # [8, 64, 3, 3]
    w9 = np.concatenate([wp, wp.sum(axis=0, keepdims=True)])  # [9, 64, 3, 3]

    w6 = np.zeros((128, 54), np.float16)
    for dj in range(3):
        # pass1: partitions 0:64 multiply tap di=0, 64:128 tap di=1
        w6[0:64, 9 * dj : 9 * dj + 9] = w9[:, :, 0, dj].T.astype(np.float16)
        w6[64:128, 9 * dj : 9 * dj + 9] = w9[:, :, 1, dj].T.astype(np.float16)
        # pass2: tap di=2 weights replicated in both halves
        w6[0:64, 27 + 9 * dj : 36 + 9 * dj] = w9[:, :, 2, dj].T.astype(np.float16)
        w6[64:128, 27 + 9 * dj : 36 + 9 * dj] = w9[:, :, 2, dj].T.astype(np.float16)

    bt9 = np.concatenate([bt, [bt.sum()]]).astype(np.float32)  # [9]
    b9x = np.zeros((128, 1), np.float32)
    for g in range(4):
        b9x[32 * g : 32 * g + 9, 0] = bt9

    # ssel cols 0:9 (bank A): group-g channel rows -> S row g; cols 9:18
    # (bank B): -> S row 4+g. Col 8 is a dummy 9th output (the dst-partition
    # ISA check wants the same 9-wide shape as the conv matmuls); it gets a
    # copy of S row 0 so its reciprocal stays finite.
    ssel = np.zeros((128, 18), np.float16)
    for g in range(4):
        ssel[32 * g : 32 * g + 8, g] = 1.0
        ssel[32 * g : 32 * g + 8, 9 + 4 + g] = 1.0
    ssel[0:8, 8] = 1.0
    # bsel, per phase ph (column block 256*ph), contracting all 128 R rows:
    # R row 32ph+g -> rows 32g+c (+1 for c<8, -1 for c=8) for bank A (h=0,
    # cols +0:128) and R row 32ph+4+g likewise for bank B (cols +128:256).
    # All other rows have zero weight.
    bsel = np.zeros((128, 1024), np.float32)
    for ph in range(4):
        for g in range(4):
            bsel[32 * ph + g, 256 * ph + 32 * g : 256 * ph + 32 * g + 8] = 1.0
            bsel[32 * ph + g, 256 * ph + 32 * g + 8] = -1.0
            bsel[32 * ph + 4 + g, 256 * ph + 128 + 32 * g : 256 * ph + 128 + 32 * g + 8] = 1.0
            bsel[32 * ph + 4 + g, 256 * ph + 128 + 32 * g + 8] = -1.0
    return w6, b9x, ssel, bsel


TRACE = False
LAST_EXEC_NS = None


def kernel(feature, conv_w, gamma, beta, mean, var, kernel_size):
    global LAST_EXEC_NS
    install()
    if TRACE:
        install_ntff_shim()

    from concourse.bass_utils import run_bass_kernel_spmd

    feature = np.asarray(feature, np.float32)
    conv_w = np.asarray(conv_w, np.float32)
    gamma = np.asarray(gamma, np.float32)
    beta = np.asarray(beta, np.float32)
    mean = np.asarray(mean, np.float32)
    var = np.asarray(var, np.float32)

    w6, b9x, ssel, bsel = make_consts(conv_w, gamma, beta, mean, var)

    # padded feature [B, C, H+3, W+2] in f16 (one extra zero row at the
    # bottom so the unused h1 slot 17 of the last block stays in range)
    fpad = np.zeros((B, C, H + 3, WP), np.float16)
    fpad[:, :, 1 : H + 1, 1 : W + 1] = feature

    in_maps = []
    for core in range(8):
        b, half = core // 2, core % 2
        h0 = half * HALF
        slab2 = np.empty((128, NBLK, SLOT * WP), np.float16)
        for k in range(NBLK):
            base = h0 + TR * k  # fpad row of slot 0 for the h0 half
            slab2[0:64, k, :] = fpad[b, :, base : base + SLOT, :].reshape(C, -1)
            slab2[64:128, k, :] = fpad[b, :, base + 1 : base + 1 + SLOT, :].reshape(
                C, -1
            )
        in_maps.append(
            {
                "slab": slab2,
                "w6": w6,
                "b9x": b9x,
                "ssel": ssel,
                "bsel": bsel,
            }
        )

    nc = build_nc()
    res = run_bass_kernel_spmd(nc, in_maps, core_ids=list(range(8)), trace=TRACE)
    LAST_EXEC_NS = res.exec_time_ns

    out_full = np.zeros((B, 9, H + 2, WP), np.float32)
    for core in range(8):
        b, half = core // 2, core % 2
        h0 = half * HALF
        r = np.asarray(res.results[core]["out"], dtype=np.float32)  # [9, 176, 1216]
        for p in range(9):
            i, j = p // 3, p % 3
            plane = r[p]
            if p == 4:
                plane = 1.0 + plane
            out_full[b, p, h0 + i : h0 + HALF + i, j : j + W] = plane
    return out_full
